# revision 56
# baseline (speedup 1.0000x reference)
"""Multi-head attention (B=2, S=2048, D=1024, H=16, Hd=64) on 8 trn2 cores.

Sharding: batch x head-group. Core c handles batch c//4 and heads
[4*(c%4), 4*(c%4)+4). Each core computes its heads' Q/K/V projections,
the masked softmax attention for those heads, and a row-parallel partial
of the output projection. Host sums the 4 partials per batch, divides by
the V-path scale (32) and adds the analytic bias terms (bv @ Wo.T + bo).

Precision plan (validated host-side, maxrel ~4.8e-3 vs limit 2e-2):
- Q/K projections and scores: fp8e4 DoubleRow (0.5 PE cycles/column).
  The host permutes W rows so each 128-partition projection chunk holds
  all 4 heads' contraction half (head h at partitions 32h+p); evictions
  stay partition-preserving and q/k land directly in the [32,2]
  DoubleRow score layout. Softmax damps the fp8 q/k error.
- V projection: split-precision fp8 DoubleRow: v*32 = x8@(32Wv)8
  + xlo8@(32Wv)8 + x8@(32Wv)lo8. The 32x scale keeps the residual
  weights out of e4m3's subnormal range; host divides partials by 32.
- exp: fp8e4 probabilities for q-chunks g<3 (tight-packed key-chunk
  pairs feeding DoubleRow PV, pair padding zeroed on Pool); fp16 for
  g=3 whose few-key queries need accurate V (fp16 pt/v16).
- Output projection: fp16; partials stored fp16.

Schedule (ACT-bound: exp is ~58us of ScalarE work; pack it):
- Units alternate head-pair within each q-chunk round, g descending:
  (hp0,g3),(hp1,g3),(hp0,g2),... so the exp stream never drains while
  PE runs interleaved projection / output work fed from per-pair hooks
  (which fire AFTER each pair's score matmuls, so exp is never queued
  behind the interleaved PE work).
- Normalization: reciprocal + unnormalized PV eviction (DVE) at unit
  end (frees the pv banks early); at the NEXT unit's first hook a
  rank-1 PE matmul broadcasts 1/r into a scratch-PSUM tile whose rows
  line up with attnT's head layout, so ONE in-place DVE multiply
  (SBUF x PSUM) normalizes both heads.
- One shared 2-slot scratch PSUM pool serves projection evictions,
  1/r broadcasts and output-projection accumulation, so PSUM fits:
  scores 4 banks (2x double-buffered) + PV 2 + scratch 2 = 8.
- V is projected two key chunks per scratch tile (one DVE eviction
  fills a whole v8 pair tile); gpsimd cannot touch PSUM, so all PSUM
  evictions ride DVE except ones placed into measured ACT idle holes
  (K/Q-s3 + 3 of 4 s2 projection chunks, v16 copies were tried and
  reverted, tail PV copies + e0 output evictions).
- Output projection tiles pop from a queue inside later units' hooks
  (DVE evictions); the final chunk is emitted inside the last unit's
  fin: per-sti normalize multiplies launch each tile immediately, tiles
  borrow the idle score/pv banks, and evictions split ACT/DVE.
- PE p-state warm-up matmuls burn the initial DMA wait; input DMAs are
  batched wide (HWDGE costs ~625ns per DMA) and ordered by first use.

Mask: reference keeps the *upper* triangle (key >= query); widened
affine_selects zero both the triangle and the beyond-width zone of
tight-packed diagonal pairs.

Softmax skips max-subtraction: |qk|/32 < ~1 so exp is safe.
"""
import contextlib

import os as _os

_jp = _os.environ.get("JAX_PLATFORMS", "")
if _jp and "axon" not in _jp:
    _os.environ["JAX_PLATFORMS"] = "axon," + _jp

import numpy as np
import ml_dtypes

import concourse.bass as bass
import concourse.tile as tile
from concourse import bacc, mybir
from concourse.bass_utils import run_bass_kernel_spmd

F32 = mybir.dt.float32
F16 = mybir.dt.float16
F8 = mybir.dt.float8e4
DR = mybir.MatmulPerfMode.DoubleRow
E4 = ml_dtypes.float8_e4m3

B = 2
S = 2048
D = 1024
HD = 64
N_CORES = 8
HEADS_PER_CORE = 4
DSL = HEADS_PER_CORE * HD  # 256 projection columns per core
P = 128
NKC = S // P  # 16 key chunks
NST = S // P  # 16 seq tiles
NCP = 4  # DoubleRow contraction-pair chunks over D (4 x (128*2))
QCH = 512
NQC = S // QCH  # 4

SCALE = 1.0 / np.sqrt(np.float32(D))  # 1/32
VS = np.float32(32.0)  # V-path scale (host divides partials)


def _build_kernel(nc: bass.Bass, repeat: int = 1):
    xq8 = nc.dram_tensor("xq8", (P, NCP, 2, S), F8, kind="ExternalInput").ap()
    xlo8 = nc.dram_tensor("xlo8", (P, NCP, 2, S), F8, kind="ExternalInput").ap()
    wqk8 = nc.dram_tensor(
        "wqk8", (P, 2, 2, NCP, 2, P), F8, kind="ExternalInput"
    ).ap()
    wv8 = nc.dram_tensor("wv8", (P, 2, NCP, 2, DSL), F8, kind="ExternalInput").ap()
    woT = nc.dram_tensor("woT", (DSL, D), F16, kind="ExternalInput").ap()
    bqkT = nc.dram_tensor("bqkT", (DSL, 2), F32, kind="ExternalInput").ap()
    bqk8 = nc.dram_tensor("bqk8", (1, 2, 2, P), F8, kind="ExternalInput").ap()
    outp = nc.dram_tensor("outp", (S, D), F16, kind="ExternalOutput").ap()

    with tile.TileContext(nc) as tc:
        for _ in range(repeat):
            _emit(tc, nc, xq8, xlo8, wqk8, wv8, woT, bqkT, bqk8, outp)
    nc.compile()
    return nc


def _emit(tc, nc, xq8, xlo8, wqk8, wv8, woT, bqkT, bqk8, outp):
    ctx = contextlib.ExitStack()

    persist = ctx.enter_context(tc.tile_pool(name="persist", bufs=1))

    # q/k in fp8 DoubleRow layout: head hc at partitions 32hc..32hc+32,
    # contraction halves (d 0-31 / 32-63) interleaved on the free axis
    qdr_sb = persist.tile([P, 2, S], F8, tag="qdr", name="qdr")
    kdr_sb = persist.tile([P, 2, S], F8, tag="kdr", name="kdr")
    # v8 pair tile t holds key chunks (2t+1, 2t) at i=(0,1): [p, i, hc, hd+1]
    v8_sb = [
        persist.tile([P, 2, HEADS_PER_CORE, 80], F8, tag=f"v8{t}", name=f"v8{t}")
        for t in range(NST // 2)
    ]
    # fp16 V copies for the g=3 diagonal (key chunks 12-15)
    v16_sb = [
        persist.tile([P, HEADS_PER_CORE, HD + 1], F16, tag=f"v16{i}", name=f"v16{i}")
        for i in range(4)
    ]
    attnt_sb = [
        persist.tile([P, S], F16, tag=f"attnt{j}", name=f"attnt{j}") for j in range(2)
    ]
    # head h's 1/rowsum lives at partition 32*h (matmul base-partition rule)
    rinv_sb = persist.tile([P, S], F16, tag="rinv", name="rinv")
    wot_sb = [
        persist.tile([P, D], F16, tag=f"wot{j}", name=f"wot{j}") for j in range(2)
    ]
    bias_sb = persist.tile([P, 2, 2], F32, tag="bias", name="bias")  # [d%128, j, proj]
    bqk8_sb = persist.tile([1, 2, 2, P], F8, tag="bqk8", name="bqk8")
    ones8_sb = persist.tile([1, QCH], F8, tag="ones8", name="ones8")
    ones64_sb = persist.tile([P, HD], F16, tag="ones64", name="ones64")
    # early Pool memsets: warm-up inputs + PV ones columns
    nc.gpsimd.memset(ones8_sb[:], 1.0)
    nc.gpsimd.memset(ones64_sb[:], 1.0)
    nc.gpsimd.memset(rinv_sb[0:1, 0:QCH], 0.0)  # warm-up rhs

    xq8_sb = persist.tile([P, NCP, 2, S], F8, tag="xq8", name="xq8")
    xlo8_sb = persist.tile([P, NCP, 2, S], F8, tag="xlo8", name="xlo8")
    wqk8_sb = persist.tile([P, 2, 2, NCP, 2, P], F8, tag="wqk8", name="wqk8")
    wv8_sb = persist.tile([P, 2, NCP, 2, DSL], F8, tag="wv8", name="wv8")

    for t in range(NST // 2):
        nc.gpsimd.memset(v8_sb[t][:, :, :, HD : HD + 1], 1.0)
    for i in range(4):
        nc.vector.memset(v16_sb[i][:, :, HD : HD + 1], 1.0)

    st_psum = ctx.enter_context(tc.tile_pool(name="st_psum", bufs=2, space="PSUM"))
    pv_psum = ctx.enter_context(tc.tile_pool(name="pv_psum", bufs=2, space="PSUM"))
    sc_psum = ctx.enter_context(tc.tile_pool(name="sc_psum", bufs=2, space="PSUM"))
    pt8_pool = ctx.enter_context(tc.tile_pool(name="pt8", bufs=10))
    pt16_pool = ctx.enter_context(tc.tile_pool(name="pt16", bufs=6))
    out_pool = ctx.enter_context(tc.tile_pool(name="outp_sb", bufs=6))

    def _outproj_sti(sti, tail=False, alt=False):
        ob = out_pool.tile([P, D], F16, tag="ob", name="ob")
        # tail tiles borrow the score pool (4 banks, idle once the exp
        # stream ends) so the broadcast tile never gates the pipeline;
        # alt tiles use the two freed pv banks for a third buffer.
        opf = (
            st_psum.tile([P, 2 * QCH], F32, tag="st", name="opf")
            if tail and not alt else None
        )
        for e in range(2):
            esl = slice(e * QCH, (e + 1) * QCH)
            if tail and alt:
                op = pv_psum.tile([P, QCH], F32, tag="pv", name="opa")[:]
            elif tail:
                op = opf[:, esl]
            else:
                op = sc_psum.tile([P, QCH], F32, tag="sc", name="op")[:]
            for j in range(2):
                nc.tensor.matmul(
                    op,
                    lhsT=attnt_sb[j][:, sti * P : (sti + 1) * P],
                    rhs=wot_sb[j][:, e * QCH : (e + 1) * QCH],
                    start=(j == 0),
                    stop=(j == 1),
                )
            if tail:
                # ACT is idle once the exp stream ends; split ACT/DVE
                if e == 0:
                    nc.scalar.copy(ob[:, esl], op)
                else:
                    nc.vector.tensor_copy(ob[:, esl], op)
            else:
                nc.vector.tensor_copy(ob[:, esl], op)
        # one DMA per tile: HWDGE issue (625ns, single slot) is the tail's
        # closing serializer
        nc.sync.dma_start(out=outp[sti * P : (sti + 1) * P, :], in_=ob[:])

    def _tri(pt, lo, w):
        # keep iff p >= (col - lo) over columns [lo, lo+w)
        nc.gpsimd.affine_select(
            out=pt[:, lo : lo + w],
            in_=pt[:, lo : lo + w],
            compare_op=mybir.AluOpType.is_ge,
            fill=0.0,
            base=0,
            channel_multiplier=1,
            pattern=[[-1, w]],
        )

    def _attn_g(hp, g, interleave=None):
        # one (head-pair, q-chunk) unit; local heads 2*hp, 2*hp+1
        kjs = list(range(NKC - 1, 4 * g - 1, -1))  # descending
        npairs = len(kjs) // 2
        gq = g * QCH
        pv = [
            pv_psum.tile([P, QCH], F32, tag="pv", name=f"pv{h}")
            for h in range(2)
        ]
        for kp in range(npairs):
            kj0, kj1 = kjs[2 * kp], kjs[2 * kp + 1]
            diag = kj1 - 4 * g <= 3  # pair inside the block-diagonal
            stp = [
                st_psum.tile([P, 2 * QCH], F32, tag="st", name=f"stp{h}")
                for h in range(2)
            ]
            if g < 3:
                # fp8 path: DoubleRow PV reads [128, 2{N}, N]; diag pairs are
                # tight-packed [0:w0][w0:w0+w1] with the tail of the i1
                # region zeroed by a Pool memset (exp/scores skip it).
                N = 256 if (diag and kj1 - 4 * g == 0) else QCH
                w0 = min(P * (kj0 - 4 * g + 1), N) if diag else N
                w1 = min(P * (kj1 - 4 * g + 1), N) if diag else N
                for i, kj, off, w in ((0, kj0, 0, w0), (1, kj1, w0, w1)):
                    for h in range(2):
                        row = slice(32 * (2 * hp + h), 32 * (2 * hp + h) + 32)
                        nc.tensor.matmul(
                            stp[h][:, off : off + w],
                            lhsT=kdr_sb[row, :, kj * P : (kj + 1) * P],
                            rhs=qdr_sb[row, :, gq : gq + w],
                            start=True,
                            stop=True,
                            perf_mode=DR,
                            tile_position=(32 * (2 * hp + h), 0),
                        )
                # hook after the scores: ACT's exp is never held behind the
                # interleaved projection/output work fed to PE here
                if interleave is not None:
                    interleave(kp)
                for h in range(2):
                    pt = pt8_pool.tile([P, 2 * QCH], F8, tag="pt8", name="pt8")
                    if diag:
                        nc.gpsimd.memset(pt[:, w0 + w1 : 2 * N], 0.0)
                    nc.scalar.activation(
                        pt[:, 0 : w0 + w1],
                        stp[h][:, 0 : w0 + w1],
                        mybir.ActivationFunctionType.Exp,
                        scale=float(SCALE),
                    )
                    if diag:
                        _tri(pt, w0 - P, P)
                        _tri(pt, w0 + w1 - P, P)
                    hc = 2 * hp + h
                    nc.tensor.matmul(
                        pv[h][0 : HD + 1, 0:N],
                        lhsT=v8_sb[kj1 // 2][:, :, hc, 0 : HD + 1],
                        rhs=pt[:, 0 : 2 * N].rearrange("p (i n) -> p i n", i=2),
                        start=(kp == 0),
                        stop=(kp == npairs - 1),
                        perf_mode=DR,
                    )
            else:
                # g=3 fp16 path, tight packing [0:w0][w0:w0+w1]
                w0 = P * (kj0 - 4 * g + 1)
                w1 = P * (kj1 - 4 * g + 1)
                for i, kj, off, w in ((0, kj0, 0, w0), (1, kj1, w0, w1)):
                    for h in range(2):
                        row = slice(32 * (2 * hp + h), 32 * (2 * hp + h) + 32)
                        nc.tensor.matmul(
                            stp[h][:, off : off + w],
                            lhsT=kdr_sb[row, :, kj * P : (kj + 1) * P],
                            rhs=qdr_sb[row, :, gq : gq + w],
                            start=True,
                            stop=True,
                            perf_mode=DR,
                            tile_position=(32 * (2 * hp + h), 0),
                        )
                if interleave is not None:
                    interleave(kp)
                for h in range(2):
                    pt = pt16_pool.tile([P, 896], F16, tag="pt16", name="pt16")
                    nc.scalar.activation(
                        pt[:, 0 : w0 + w1],
                        stp[h][:, 0 : w0 + w1],
                        mybir.ActivationFunctionType.Exp,
                        scale=float(SCALE),
                    )
                    _tri(pt, w0 - P, P)
                    _tri(pt, w0 + w1 - P, P)
                    hc = 2 * hp + h
                    for i, kj, off, w in ((0, kj0, 0, w0), (1, kj1, w0, w1)):
                        nc.tensor.matmul(
                            pv[h][0 : HD + 1, 0:w],
                            lhsT=v16_sb[kj - 12][:, hc, :],
                            rhs=pt[:, off : off + w],
                            start=(kj == NKC - 1),
                            stop=(kj == 4 * g),
                        )

        # normalization: 1/rowsum + unnormalized PV eviction at unit end
        # (the copies free the pv slots early; the last unit evicts via the
        # then-idle ACT engine). The multiply against the rank-1 PE
        # broadcast of 1/r is deferred to the NEXT unit's first hook so PE
        # never stalls on the reciprocal.
        gsl = slice(gq, gq + QCH)
        last = (hp, g) == (1, 0)
        for h in range(2):
            hc = 2 * hp + h
            with nc.allow_low_precision(reason="fp16 1/rowsum, rel err ~5e-4"):
                nc.vector.reciprocal(
                    out=rinv_sb[32 * hc : 32 * hc + 1, gsl],
                    in_=pv[h][HD : HD + 1, :],
                )
            cp_eng = nc.scalar.copy if last else nc.vector.tensor_copy
            cp_eng(
                attnt_sb[hp][HD * h : HD * (h + 1), gsl],
                pv[h][0:HD, :],
            )

        def fin(last=last):
            # bc rows 0-63/64-127 line up with attnT's head layout, so one
            # in-place DVE multiply (SBUF x PSUM) normalizes both heads.
            bc = sc_psum.tile([P, QCH], F32, tag="sc", name="bc")
            for h in range(2):
                hc = 2 * hp + h
                nc.tensor.matmul(
                    bc[HD * h : HD * (h + 1), :],
                    lhsT=ones64_sb[32 * hc : 32 * hc + 1, :],
                    rhs=rinv_sb[32 * hc : 32 * hc + 1, gsl],
                    start=True,
                    stop=True,
                    tile_position=(32 * hc, HD * h),
                )
            if last:
                # tail: normalize per-sti and launch each output tile
                # immediately so the final projection pipelines behind it.
                for sti in range(4):
                    psl = slice(gq + sti * P, gq + (sti + 1) * P)
                    nc.vector.tensor_mul(
                        attnt_sb[hp][:, psl],
                        attnt_sb[hp][:, psl],
                        bc[:, sti * P : (sti + 1) * P],
                    )
                    _outproj_sti(g * 4 + sti, tail=True, alt=(sti in (1, 3)))
            else:
                nc.vector.tensor_mul(
                    attnt_sb[hp][:, gsl], attnt_sb[hp][:, gsl], bc[:]
                )

        return fin

    # --- DMAs, ordered by first use (xq8 sch3 + wqk gate the first proj) --
    nc.sync.dma_start(
        out=xq8_sb[:, :, :, 3 * QCH : S], in_=xq8[:, :, :, 3 * QCH : S]
    )
    nc.sync.dma_start(out=wqk8_sb[:, :, 1], in_=wqk8[:, :, 1, :, :, :])
    nc.sync.dma_start(out=wqk8_sb[:, :, 0], in_=wqk8[:, :, 0, :, :, :])
    nc.sync.dma_start(
        out=bias_sb[:],
        in_=bqkT.rearrange("(j p) t -> p j t", j=2),
    )
    nc.sync.dma_start(out=bqk8_sb[:], in_=bqk8[:, :, :, :])
    nc.sync.dma_start(out=wv8_sb[:], in_=wv8[:, :, :, :, :])
    # ordered by first use: v15-12 (xlo8 s3) during the g3 round, then
    # Q/K-s2, the sch2 V chunks, and the rest descending
    nc.sync.dma_start(
        out=xlo8_sb[:, :, :, 3 * QCH : S], in_=xlo8[:, :, :, 3 * QCH : S]
    )
    nc.sync.dma_start(
        out=xq8_sb[:, :, :, 2 * QCH : 3 * QCH], in_=xq8[:, :, :, 2 * QCH : 3 * QCH]
    )
    nc.sync.dma_start(
        out=xlo8_sb[:, :, :, 2 * QCH : 3 * QCH], in_=xlo8[:, :, :, 2 * QCH : 3 * QCH]
    )
    nc.sync.dma_start(
        out=xq8_sb[:, :, :, 1 * QCH : 2 * QCH], in_=xq8[:, :, :, 1 * QCH : 2 * QCH]
    )
    nc.sync.dma_start(
        out=xlo8_sb[:, :, :, 0 : 2 * QCH], in_=xlo8[:, :, :, 0 : 2 * QCH]
    )
    nc.sync.dma_start(
        out=xq8_sb[:, :, :, 0 : 1 * QCH], in_=xq8[:, :, :, 0 : 1 * QCH]
    )
    for j in range(2):
        nc.sync.dma_start(out=wot_sb[j][:], in_=woT[j * P : (j + 1) * P, :])

    # PE p-state warm-up: the cost model runs PE at 0.65-1.2GHz until it
    # has been continuously busy ~3us; burn the initial DMA wait on dummy
    # rank-1 matmuls so the first projections run at full clock.
    warm = sc_psum.tile([P, QCH], F32, tag="sc", name="warm")
    for w in range(6):
        nc.tensor.matmul(
            warm[0:1, :],
            lhsT=ones64_sb[0:1, 0:1],
            rhs=rinv_sb[0:1, 0:QCH],
            start=True,
            stop=True,
        )

    def qk_proj(proj, half, sch, act=False):
        # q/k in DoubleRow layout: the host permutes W rows so PSUM
        # partition 32h+p is head h, contraction-dim 32*half+p; the
        # eviction is partition-preserving into qdr/kdr[:, half, :].
        # act=True folds the bias in as a rank-1 PE matmul and evicts
        # via ACT (idle in the early window).
        dst = qdr_sb if proj == 0 else kdr_sb
        ps = sc_psum.tile([P, QCH], F32, tag="sc", name="pp")
        for cp in range(NCP):
            nc.tensor.matmul(
                ps[:],
                lhsT=wqk8_sb[:, half, proj, cp, :, :],
                rhs=xq8_sb[:, cp, :, sch * QCH : (sch + 1) * QCH],
                start=(cp == 0),
                stop=(cp == NCP - 1) and not act,
                perf_mode=DR,
            )
        dsl_ = dst[:, half, sch * QCH : (sch + 1) * QCH]
        if act:
            nc.tensor.matmul(
                ps[:],
                lhsT=bqk8_sb[0:1, half, proj, :],
                rhs=ones8_sb[0:1, :],
                start=False,
                stop=True,
            )
            nc.scalar.copy(dsl_, ps[:])
        else:
            nc.vector.tensor_scalar_add(
                dsl_, ps[:], bias_sb[:, half, proj : proj + 1]
            )

    def v_proj(t):
        # V*32 for pair tile t (key chunks 2t+1, 2t) in one scratch tile:
        # halves the scratch-slot round-trips and evicts both chunks with
        # a single DVE copy. Split-fp8: x8@(32Wv)8 + xlo8@(32Wv)8
        # + x8@(32Wv)lo8; chunk 2t+1 -> i=0, 2t -> i=1.
        ps = sc_psum.tile([P, QCH], F32, tag="sc", name="ppv")
        terms = [(xq8_sb, 0), (xlo8_sb, 0), (xq8_sb, 1)]
        for i, st in ((0, 2 * t + 1), (1, 2 * t)):
            n = 0
            for xs, hl in terms:
                for cp in range(NCP):
                    nc.tensor.matmul(
                        ps[:, i * DSL : (i + 1) * DSL],
                        lhsT=xs[:, cp, :, st * P : (st + 1) * P],
                        rhs=wv8_sb[:, hl, cp, :, :],
                        start=(n == 0),
                        stop=(n == 3 * NCP - 1),
                        perf_mode=DR,
                    )
                    n += 1
        nc.vector.tensor_copy(
            v8_sb[t][:, :, :, 0:HD],
            ps[:].rearrange("p (i h d) -> p i h d", i=2, h=HEADS_PER_CORE),
        )
        if t >= 6:
            for i, st in ((0, 2 * t + 1), (1, 2 * t)):
                nc.vector.tensor_copy(
                    v16_sb[st - 12][:, :, 0:HD],
                    ps[:, i * DSL : (i + 1) * DSL].rearrange(
                        "p (h d) -> p h d", h=HEADS_PER_CORE
                    ),
                )

    # first projections: K sch3 evicts via ACT (idle), Q sch3 via DVE;
    # Q-s0 (needed by the g0 round) follows while PE idles under g3 exps
    qk_proj(1, 0, 3, act=True)
    qk_proj(1, 1, 3, act=True)
    qk_proj(0, 0, 3)
    qk_proj(0, 1, 3)

    # unit schedule: alternate hp within each g round, g descending.
    # hooks[(hp, g)][kp] = list of callables; `fin` of the previous unit
    # is prepended to hook kp0 at emission time.
    opq = []  # output-projection tiles ready to interleave

    def op_pop():
        if opq:
            _outproj_sti(opq.pop(0))

    # units alternate hp within each q-chunk round, g descending; the
    # projections and output tiles spread over the per-pair hooks.
    hooks = {
        (0, 3): {0: [lambda: v_proj(7)],
                 1: [lambda: v_proj(6)]},
        (1, 3): {0: [lambda: qk_proj(0, 0, 2), lambda: qk_proj(0, 1, 2, act=True)],
                 1: [lambda: qk_proj(1, 0, 2, act=True),
                     lambda: qk_proj(1, 1, 2, act=True)]},
        (0, 2): {0: [lambda: qk_proj(0, 0, 1)],
                 1: [lambda: v_proj(5), lambda: qk_proj(0, 1, 1)],
                 2: [lambda: v_proj(4)],
                 3: []},
        (1, 2): {0: [lambda: qk_proj(1, 0, 1)],
                 1: [lambda: qk_proj(1, 1, 1)],
                 2: [lambda: v_proj(3)],
                 3: []},
        (0, 1): {0: [lambda: qk_proj(0, 0, 0)],
                 1: [lambda: qk_proj(0, 1, 0)],
                 2: [lambda: v_proj(2)],
                 3: [op_pop],
                 4: [op_pop],
                 5: []},
        (1, 1): {0: [lambda: qk_proj(1, 0, 0)],
                 1: [lambda: qk_proj(1, 1, 0)],
                 2: [lambda: v_proj(1)],
                 3: [op_pop],
                 4: [op_pop],
                 5: []},
        (0, 0): {0: [],
                 1: [lambda: v_proj(0)],
                 2: [op_pop],
                 3: [op_pop],
                 4: [op_pop],
                 5: [op_pop],
                 6: [op_pop], 7: [op_pop]},
        (1, 0): {0: [], 1: [op_pop], 2: [op_pop], 3: [], 4: [],
                 5: [], 6: [], 7: []},
    }

    units = [(hp, g) for g in range(NQC - 1, -1, -1) for hp in range(2)]
    prev_fin = [None]
    for hp, g in units:
        hg = hooks[(hp, g)]

        def hook(kp, hg=hg):
            # hook fns first: their DVE evictions gate qdr/kdr/v8 (the
            # score/PV inputs); the deferred fin multiplies queue after
            for fn in hg.get(kp, []):
                fn()
            if kp == 0 and prev_fin[0] is not None:
                prev_fin[0]()
                prev_fin[0] = None

        prev_fin[0] = _attn_g(hp, g, interleave=hook)
        # chunk g becomes computable once (1,g)'s fin runs (at the next
        # unit's kp0 hook); the first op_pop for it sits after that.
        if hp == 1 and g > 0:
            opq.extend(range(4 * g, 4 * g + 4))
    prev_fin[0]()  # last unit's fin emits the final chunk's tiles inline
    while opq:
        _outproj_sti(opq.pop(0), tail=True)

    ctx.close()


_NC_CACHE = None


def _get_nc():
    global _NC_CACHE
    if _NC_CACHE is None:
        nc = bacc.Bacc("TRN2", target_bir_lowering=False, debug=False)
        _NC_CACHE = _build_kernel(nc)
    return _NC_CACHE


def _dr_x(xt: np.ndarray) -> np.ndarray:
    # (D, S) e4m3 -> [128, NCP, 2, S]: d = 256*cp + 128*i + p
    return np.ascontiguousarray(xt.reshape(NCP, 2, P, S).transpose(2, 0, 1, 3))


def kernel(x, Wq, bq, Wk, bk, Wv, bv, Wo, bo):
    x = np.asarray(x, dtype=np.float32)
    Wq, bq = np.asarray(Wq, np.float32), np.asarray(bq, np.float32)
    Wk, bk = np.asarray(Wk, np.float32), np.asarray(bk, np.float32)
    Wv, bv = np.asarray(Wv, np.float32), np.asarray(bv, np.float32)
    Wo, bo = np.asarray(Wo, np.float32), np.asarray(bo, np.float32)

    nc = _get_nc()

    x8 = x.astype(E4)
    xlo8 = (x - x8.astype(np.float32)).astype(E4)

    in_maps = []
    for c in range(N_CORES):
        b = c // 4
        hg = c % 4
        hsl = slice(hg * DSL, (hg + 1) * DSL)

        # QK weights, DR layout [128, half, proj, cp, i, out]: output rows
        # permuted so PSUM partition 32h+p = head h, d = 32*half+p
        perm = (
            64 * np.repeat(np.arange(4), 32) + np.tile(np.arange(32), 4)
        )  # head-major d-low rows; +32 for the high half

        def _dr_w(Wm):
            w8 = Wm[hsl].astype(E4)  # (256, 1024)
            halves = [w8[perm + 32 * half] for half in (0, 1)]  # (128, 1024)
            return np.stack(
                [
                    h.T.reshape(NCP, 2, P, P).transpose(2, 0, 1, 3)
                    for h in halves
                ],
                axis=1,
            )  # [128, half, cp, i, out]

        wqk8 = np.ascontiguousarray(
            np.stack([_dr_w(Wq), _dr_w(Wk)], axis=2)
        )  # [128, half, proj, cp, i, out]

        # V weights, scaled by 32, hi/lo split, [128, hl, cp, i, out]
        vh = (VS * Wv[hsl]).astype(E4)  # (256, 1024)
        vlo = (VS * Wv[hsl] - vh.astype(np.float32)).astype(E4)

        def _dr_v(Vm):
            return Vm.T.reshape(NCP, 2, P, DSL).transpose(2, 0, 1, 3)

        wv8 = np.ascontiguousarray(np.stack([_dr_v(vh), _dr_v(vlo)], axis=1))

        bqk8_host = np.ascontiguousarray(
            np.stack(
                [
                    np.stack([bb[hsl][perm], bb[hsl][perm + 32]], 0)
                    for bb in (bq, bk)
                ],
                axis=1,
            ).reshape(1, 2, 2, P),
            dtype=np.float32,
        ).astype(E4)
        # note: stacked as [half, proj, row] -> need [1, half, proj, row]
        in_maps.append(
            {
                "bqk8": bqk8_host,
                "xq8": _dr_x(np.ascontiguousarray(x8[b].T)),
                "xlo8": _dr_x(np.ascontiguousarray(xlo8[b].T)),
                "wqk8": wqk8,
                "wv8": wv8,
                "woT": np.ascontiguousarray(Wo[:, hsl].T, dtype=np.float16),
                "bqkT": np.ascontiguousarray(
                    np.stack(
                        [
                            np.stack([bq[hsl][perm], bq[hsl][perm + 32]], 0),
                            np.stack([bk[hsl][perm], bk[hsl][perm + 32]], 0),
                        ],
                        axis=2,
                    ).reshape(DSL, 2),
                    dtype=np.float32,
                ),
            }
        )

    res = run_bass_kernel_spmd(
        nc, in_maps, core_ids=list(range(N_CORES)), trace=False
    )

    # host gather: sum partials per batch, unscale V path, add bias terms
    bias_term = (bv @ Wo.T + bo).astype(np.float32)  # (D,)
    out = np.empty((B, S, D), dtype=np.float32)
    for b in range(B):
        acc = res.results[4 * b]["outp"].astype(np.float32)
        for c in range(4 * b + 1, 4 * b + 4):
            acc = acc + res.results[c]["outp"].astype(np.float32)
        out[b] = acc / VS + bias_term
    return out
